# revision 1
# baseline (speedup 1.0000x reference)
"""HDModel retrieval kernel for 8x TRN2 NeuronCores.

reference:
    sims  = l2norm(hvs) @ l2norm(am).T        # [N, C] cosine sims
    preds = argmax(sims, axis=1)              # int32 [N]
    eta   = (sims[:,1]-sims[:,0])*0.25 + 0.5  # f32 [N]

Strategy (data-parallel over N, am replicated — no cross-core comms):
  - Host pre-transposes hvs -> hvsT [D, N/8] per shard and am -> amT [D, C]
    (layout staging only; all math happens on device).
  - sims are computed as raw = hvsT.T @ amT with f32r (tf32) matmuls,
    1 cyc/row on the PE at N>=256 vs fp32's 4.
  - am column norms (needed before argmax) via a bf16 ones-matmul over
    squared amT chunks; per-class scaling applied to sims rows on DVE.
  - hvs row norms (needed only for eta) via a bf16 gram matmul per n-tile;
    diagonal extracted with one DVE tensor_tensor_reduce against identity.
  - argmax via DVE max + max_index (top-8, index 0 = first-max like jnp).
  - preds/eta accumulate in [128, NT] tiles, one DMA out at the end;
    host reorders ([p, t] -> n = t*128+p).

This walrus build encodes ONE sync wait per TPB instruction; Tile attaches
several, so a post-pass splits multi-wait instructions into single-wait
same-engine NoOps (see _split_multiwait).
"""
import numpy as np
from contextlib import ExitStack

import concourse.bass as bass
import concourse.mybir as mybir
import concourse.tile as tile
from concourse.bass_utils import run_bass_kernel_spmd
from concourse.masks import make_identity

f32 = mybir.dt.float32
f32r = mybir.dt.float32r
bf16 = mybir.dt.bfloat16
u32 = mybir.dt.uint32

N_CORES = 8
N_FULL, D, C = 16384, 4096, 1024
NS = N_FULL // N_CORES          # 2048 rows per core
NT = NS // 128                  # 16 n-tiles
DCH = D // 128                  # 32 d-chunks
EPS = 1e-8


def _split_multiwait(nc):
    """Split multi-wait instructions into single-wait NoOps (walrus limit)."""
    ctr = [0]

    def mk_nop(engine, wait=None, update=None):
        ctr[0] += 1
        nop = mybir.InstNoOp(name=f"mwsplit_{ctr[0]}", ins=[], outs=[])
        nop.engine = engine
        nop.sync_info = mybir.SyncInfo(
            on_wait=[wait] if wait is not None else [],
            on_update=[update] if update is not None else [],
        )
        return nop

    for f in nc.m.functions:
        for bb in f.blocks:
            new = []
            changed = False
            for inst in bb.instructions:
                si = inst.sync_info
                if si is None:
                    new.append(inst)
                    continue
                waits = list(si.on_wait)
                updates = list(si.on_update)
                pre, post = [], []
                if len(waits) > 1:
                    pre = [mk_nop(inst.engine, wait=w) for w in waits[:-1]]
                    waits = waits[-1:]
                if len(updates) > 1 and type(inst).__name__ != "InstDMACopy":
                    post = [mk_nop(inst.engine, update=u) for u in updates[1:]]
                    updates = updates[:1]
                if pre or post:
                    inst.sync_info = mybir.SyncInfo(on_wait=waits, on_update=updates)
                    new.extend(pre)
                    new.append(inst)
                    new.extend(post)
                    changed = True
                else:
                    new.append(inst)
            if changed:
                bb.instructions = new


def build_nc():
    nc = bass.Bass()
    hvsT = nc.declare_dram_parameter("hvsT", [D, NS], f32r, isOutput=False)
    amT = nc.declare_dram_parameter("amT", [D, C], f32r, isOutput=False)
    ones_b = nc.declare_dram_parameter("ones_b", [128, 1], bf16, isOutput=False)
    ones_c = nc.declare_dram_parameter("ones_c", [1, 128], f32, isOutput=False)
    ident32 = nc.declare_dram_parameter("ident32", [128, 32], f32, isOutput=False)
    preds_o = nc.declare_dram_parameter("preds", [128, NT], u32, isOutput=True)
    eta_o = nc.declare_dram_parameter("eta", [128, NT], f32, isOutput=True)

    with tile.TileContext(nc) as tc, ExitStack() as ctx:
        const_p = ctx.enter_context(tc.tile_pool(name="const", bufs=1))
        am_p = ctx.enter_context(tc.tile_pool(name="am", bufs=1))
        sq_p = ctx.enter_context(tc.tile_pool(name="sq", bufs=3))
        hx_p = ctx.enter_context(tc.tile_pool(name="hx", bufs=2))
        hb_p = ctx.enter_context(tc.tile_pool(name="hb", bufs=2))
        ep_p = ctx.enter_context(tc.tile_pool(name="ep", bufs=2))
        acc_p = ctx.enter_context(tc.tile_pool(name="acc", bufs=1))
        ps_p = ctx.enter_context(tc.tile_pool(name="ps", bufs=2, space="PSUM"))
        psn_p = ctx.enter_context(tc.tile_pool(name="psn", bufs=1, space="PSUM"))

        # ---- constants ----
        ident = const_p.tile([128, 32], f32)
        nc.sync.dma_start(ident[:], ident32[:])
        ones_t = const_p.tile([128, 1], bf16)
        nc.sync.dma_start(ones_t[:], ones_b[:])
        ones_ct = const_p.tile([1, 128], f32)
        nc.sync.dma_start(ones_ct[:], ones_c[:])

        # ---- load amT (resident) ----
        am_tiles = []
        for dc in range(DCH):
            t = am_p.tile([128, C], f32r, tag=f"am{dc}")
            nc.sync.dma_start(t[:], amT[dc * 128:(dc + 1) * 128, :])
            am_tiles.append(t)

        # ---- am column norms: ones.T @ (amT**2), bf16 ----
        pn0 = psn_p.tile([1, 512], f32, tag="amn0")
        pn1 = psn_p.tile([1, 512], f32, tag="amn1")
        for dc in range(DCH):
            sq = sq_p.tile([128, C], bf16, tag="sq")
            nc.vector.tensor_mul(sq[:], am_tiles[dc][:].bitcast(f32),
                                 am_tiles[dc][:].bitcast(f32))
            nc.tensor.matmul(pn0[:], ones_t[:], sq[:, 0:512],
                             start=(dc == 0), stop=(dc == DCH - 1))
            nc.tensor.matmul(pn1[:], ones_t[:], sq[:, 512:C],
                             start=(dc == 0), stop=(dc == DCH - 1))

        # inv_c = 1 / max(sqrt(normsq), EPS), laid out [1, C] on partition 0
        amn = const_p.tile([1, C], f32)
        nc.scalar.sqrt(amn[:, 0:512], pn0[:])
        nc.scalar.sqrt(amn[:, 512:C], pn1[:])
        nc.vector.tensor_scalar_max(amn[:], amn[:], EPS)
        inv_c = const_p.tile([1, C], f32)
        nc.vector.reciprocal(inv_c[:], amn[:])

        # broadcast inv_c to all 128 partitions via exact fp32 ones-outer-product
        inv_cb = const_p.tile([128, C], f32)
        for h in range(2):
            bc = ps_p.tile([128, 512], f32, tag=("psA" if h == 0 else "psB"))
            nc.tensor.matmul(bc[:], ones_ct[:], inv_c[:, h * 512:(h + 1) * 512],
                             start=True, stop=True)
            nc.scalar.copy(inv_cb[:, h * 512:(h + 1) * 512], bc[:])

        # ---- accumulators ----
        preds_acc = acc_p.tile([128, NT], u32)
        eta_acc = acc_p.tile([128, NT], f32)

        # ---- main loop over n-tiles ----
        for t in range(NT):
            hx = hx_p.tile([128, D], f32r, tag="hx")
            src = hvsT[:, t * 128:(t + 1) * 128].rearrange(
                "(dc p) j -> p dc j", p=128)
            hxv = hx[:].rearrange("p (dc j) -> p dc j", j=128)
            half = DCH // 2
            nc.sync.dma_start(hxv[:, 0:half, :], src[:, 0:half, :])
            nc.sync.dma_start(hxv[:, half:DCH, :], src[:, half:DCH, :])

            hb = hb_p.tile([128, D], bf16, tag="hb")
            nc.scalar.copy(hb[:, 0:D // 2], hx[:, 0:D // 2].bitcast(f32))
            nc.scalar.copy(hb[:, D // 2:D], hx[:, D // 2:D].bitcast(f32))

            pA = ps_p.tile([128, 512], f32, tag="psA")
            pB = ps_p.tile([128, 512], f32, tag="psB")
            pG = ps_p.tile([128, 32], f32, tag="psG")
            for dc in range(DCH):
                lhs = hx[:, dc * 128:(dc + 1) * 128]
                nc.tensor.matmul(pA[:], lhs, am_tiles[dc][:, 0:512],
                                 start=(dc == 0), stop=(dc == DCH - 1))
                nc.tensor.matmul(pB[:], lhs, am_tiles[dc][:, 512:C],
                                 start=(dc == 0), stop=(dc == DCH - 1))
            # 4 col-packed 32-wide gram blocks run concurrently in the PE
            # array (tile_position col-tiling); only the diagonal is needed.
            for dc in range(DCH):
                for b in range(4):
                    sl = hb[:, dc * 128 + 32 * b:dc * 128 + 32 * (b + 1)]
                    nc.tensor.matmul(pG[32 * b:32 * (b + 1), :], sl, sl,
                                     start=(dc == 0), stop=(dc == DCH - 1),
                                     tile_position=(0, 32 * b))

            # epilogue
            sc = ep_p.tile([128, C], f32, tag="sc")
            nc.vector.tensor_mul(sc[:, 0:512], pA[:], inv_cb[:, 0:512])
            nc.vector.tensor_mul(sc[:, 512:C], pB[:], inv_cb[:, 512:C])

            dg = ep_p.tile([128, 32], f32, tag="dg")
            nsq = ep_p.tile([128, 1], f32, tag="nsq")
            nc.vector.tensor_mul(dg[:], pG[:], ident[:])
            nc.vector.reduce_sum(nsq[:], dg[:], axis=mybir.AxisListType.X)
            nrm = ep_p.tile([128, 1], f32, tag="nrm")
            nc.scalar.sqrt(nrm[:], nsq[:])
            nc.vector.tensor_scalar_max(nrm[:], nrm[:], EPS)
            inv_n = ep_p.tile([128, 1], f32, tag="invn")
            nc.vector.reciprocal(inv_n[:], nrm[:])

            mx = ep_p.tile([128, 8], f32, tag="mx")
            ix = ep_p.tile([128, 8], u32, tag="ix")
            nc.vector.max(out=mx[:], in_=sc[:])
            nc.vector.max_index(out=ix[:], in_max=mx[:], in_values=sc[:])
            nc.vector.tensor_copy(preds_acc[:, t:t + 1], ix[:, 0:1])

            d01 = ep_p.tile([128, 1], f32, tag="d01")
            nc.vector.tensor_sub(d01[:], sc[:, 1:2], sc[:, 0:1])
            nc.vector.tensor_mul(d01[:], d01[:], inv_n[:])
            nc.vector.tensor_scalar(
                out=eta_acc[:, t:t + 1], in0=d01[:], scalar1=0.25, scalar2=0.5,
                op0=mybir.AluOpType.mult, op1=mybir.AluOpType.add)

        nc.sync.dma_start(preds_o[:], preds_acc[:])
        nc.sync.dma_start(eta_o[:], eta_acc[:])

    _split_multiwait(nc)
    return nc


_CACHE = {}


def kernel(hvs: np.ndarray, am: np.ndarray):
    hvs = np.asarray(hvs, dtype=np.float32)
    am = np.asarray(am, dtype=np.float32)
    assert hvs.shape == (N_FULL, D) and am.shape == (C, D)

    if "nc" not in _CACHE:
        _CACHE["nc"] = build_nc()
    nc = _CACHE["nc"]

    amT = np.ascontiguousarray(am.T)                      # [D, C]
    import ml_dtypes
    ones_b = np.ones((128, 1), dtype=ml_dtypes.bfloat16)
    ones_c = np.ones((1, 128), dtype=np.float32)
    ident32 = np.zeros((128, 32), dtype=np.float32)
    for b in range(4):
        ident32[32 * b:32 * (b + 1), :] = np.eye(32, dtype=np.float32)

    in_maps = []
    for r in range(N_CORES):
        shard = hvs[r * NS:(r + 1) * NS]                  # [NS, D]
        hvsT = np.ascontiguousarray(shard.T)              # [D, NS]
        in_maps.append({"hvsT": hvsT, "amT": amT, "ones_b": ones_b,
                        "ones_c": ones_c, "ident32": ident32})

    res = run_bass_kernel_spmd(nc, in_maps, core_ids=list(range(N_CORES)))

    preds = np.empty(N_FULL, dtype=np.int32)
    eta = np.empty(N_FULL, dtype=np.float32)
    for r in range(N_CORES):
        p = res.results[r]["preds"]                       # [128, NT] u32
        e = res.results[r]["eta"]                         # [128, NT] f32
        preds[r * NS:(r + 1) * NS] = p.T.ravel().astype(np.int32)
        eta[r * NS:(r + 1) * NS] = e.T.ravel()
    return preds, eta



# revision 12
# speedup vs baseline: 1.1823x; 1.1823x over previous
"""HDModel retrieval kernel for 8x TRN2 NeuronCores.

reference:
    sims  = l2norm(hvs) @ l2norm(am).T        # [N, C] cosine sims
    preds = argmax(sims, axis=1)              # int32 [N]
    eta   = (sims[:,1]-sims[:,0])*0.25 + 0.5  # f32 [N]

Strategy (data-parallel over N, am replicated — no cross-core comms):
  - Host pre-transposes hvs -> hvsT [D, N/8] per shard and am -> amT [D, C]
    (layout staging only; all math happens on device).
  - sims via f32r (tf32) matmuls, 1 cyc/row on the PE at ap>=256.
  - Each 128-row n-tile is computed as TWO half-C passes (classes 0:512
    then 512:1024) so one pass needs only ONE PSUM bank. During the am
    load window a 4-tile "wave" accumulates chunk-paced as am chunks
    arrive, hiding the am DMA behind sims matmuls (the baseline idled
    the PE ~50us waiting for am + am norms before the main loop).
  - am column norms: DVE squares each am chunk (bf16); reversed-operand
    matmuls (stationary=sq chunk, moving=ones, ap=1 => ~0 PE engine
    time) accumulate per-class norm^2 in column layout [c,1]; a
    transpose matmul + exact fp32 ones outer-product broadcasts
    1/max(norm,eps) to [128, C].
  - hvs row norms (needed only for eta): ACT squares hx pieces (bf16),
    reversed ones-matmuls accumulate norm^2 per tile, ~0 PE time.
  - argmax: DVE max + max_index per half (first-max like jnp), then a
    cross-half compare (strict > keeps the lower index on ties).
  - preds/eta accumulate in [128, NT] tiles, one DMA out at the end;
    host reorders ([p, t] -> n = t*128+p).

This walrus build encodes ONE sync wait per TPB instruction; Tile attaches
several, so a post-pass splits multi-wait instructions into single-wait
same-engine NoOps (see _split_multiwait).
"""
import numpy as np
from contextlib import ExitStack

import concourse.bass as bass
import concourse.mybir as mybir
import concourse.tile as tile
from concourse.bass_utils import run_bass_kernel_spmd

f32 = mybir.dt.float32
f32r = mybir.dt.float32r
bf16 = mybir.dt.bfloat16
u32 = mybir.dt.uint32

N_CORES = 8
N_FULL, D, C = 16384, 4096, 1024
NS = N_FULL // N_CORES          # 2048 rows per core
NT = NS // 128                  # 16 n-tiles
DCH = D // 128                  # 32 d-chunks
CH = C // 2                     # 512 classes per half
WAVE = 4                        # tiles processed chunk-paced during am load
EPS = 1e-8


def _split_multiwait(nc):
    """Split multi-wait instructions into single-wait NoOps (walrus limit)."""
    ctr = [0]

    def mk_nop(engine, wait=None, update=None):
        ctr[0] += 1
        nop = mybir.InstNoOp(name=f"mwsplit_{ctr[0]}", ins=[], outs=[])
        nop.engine = engine
        nop.sync_info = mybir.SyncInfo(
            on_wait=[wait] if wait is not None else [],
            on_update=[update] if update is not None else [],
        )
        return nop

    for f in nc.m.functions:
        for bb in f.blocks:
            new = []
            changed = False
            for inst in bb.instructions:
                si = inst.sync_info
                if si is None:
                    new.append(inst)
                    continue
                waits = list(si.on_wait)
                updates = list(si.on_update)
                pre, post = [], []
                if len(waits) > 1:
                    pre = [mk_nop(inst.engine, wait=w) for w in waits[:-1]]
                    waits = waits[-1:]
                if len(updates) > 1 and type(inst).__name__ != "InstDMACopy":
                    post = [mk_nop(inst.engine, update=u) for u in updates[1:]]
                    updates = updates[:1]
                if pre or post:
                    inst.sync_info = mybir.SyncInfo(on_wait=waits, on_update=updates)
                    new.extend(pre)
                    new.append(inst)
                    new.extend(post)
                    changed = True
                else:
                    new.append(inst)
            if changed:
                bb.instructions = new


def build_nc():
    nc = bass.Bass()
    hvsT = nc.declare_dram_parameter("hvsT", [D, NS], f32r, isOutput=False)
    amT = nc.declare_dram_parameter("amT", [D, C], f32r, isOutput=False)
    ones_b = nc.declare_dram_parameter("ones_b", [128, 1], bf16, isOutput=False)
    ones_r = nc.declare_dram_parameter("ones_r", [1, 128], f32, isOutput=False)
    ident = nc.declare_dram_parameter("ident", [128, 128], f32, isOutput=False)
    preds_o = nc.declare_dram_parameter("preds", [128, NT], u32, isOutput=True)
    eta_o = nc.declare_dram_parameter("eta", [128, NT], f32, isOutput=True)

    with tile.TileContext(nc) as tc, ExitStack() as ctx:
        const_p = ctx.enter_context(tc.tile_pool(name="const", bufs=1))
        am_p = ctx.enter_context(tc.tile_pool(name="am", bufs=1))
        hx_p = ctx.enter_context(tc.tile_pool(name="hx", bufs=4))
        sqa_p = ctx.enter_context(tc.tile_pool(name="sqa", bufs=2))
        sqh_p = ctx.enter_context(tc.tile_pool(name="sqh", bufs=4))
        sc_p = ctx.enter_context(tc.tile_pool(name="sc", bufs=1))
        ep_p = ctx.enter_context(tc.tile_pool(name="ep", bufs=2))
        acc_p = ctx.enter_context(tc.tile_pool(name="acc", bufs=1))
        wv_p = ctx.enter_context(tc.tile_pool(name="wv", bufs=5, space="PSUM"))
        nrm_p = ctx.enter_context(tc.tile_pool(name="nrm", bufs=1, space="PSUM"))
        tr_p = ctx.enter_context(tc.tile_pool(name="tr", bufs=1, space="PSUM"))
        bc_p = ctx.enter_context(tc.tile_pool(name="bc", bufs=1, space="PSUM"))

        # ---- constants ----
        ones_t = const_p.tile([128, 1], bf16)
        nc.sync.dma_start(ones_t[:], ones_b[:])
        ones_ct = const_p.tile([1, 128], f32)
        nc.sync.dma_start(ones_ct[:], ones_r[:])
        ident_t = const_p.tile([128, 128], f32)
        nc.sync.dma_start(ident_t[:], ident[:])

        # ---- persistent tiles ----
        # norm bank: cols 0..3 am-norm^2 A-blocks, 4..7 B-blocks, 8+t hvs
        # norm^2 of tile t
        nrm = nrm_p.tile([128, 8 + NT], f32)
        preds_acc = acc_p.tile([128, NT], u32)
        eta_acc = acc_p.tile([128, NT], f32)
        mx_acc = acc_p.tile([128, 8 * NT], f32)      # A-half top-8 per tile
        inv_cb = acc_p.tile([128, C], f32)           # bcast 1/am-norm

        am_tiles = {}   # (half, k) -> [128, CH] tile
        hx_tiles = {}   # t -> [128, D] tile

        def dma_am(half, k):
            t = am_p.tile([128, CH], f32r, tag=f"am{half}_{k}")
            nc.sync.dma_start(t[:], amT[k * 128:(k + 1) * 128,
                                        half * CH:(half + 1) * CH])
            am_tiles[(half, k)] = t

        def dma_hx_quarter(t, q):
            if t not in hx_tiles:
                hx_tiles[t] = hx_p.tile([128, D], f32r, tag="hx",
                                        name=f"hx{t}")
            hx = hx_tiles[t]
            src = hvsT[:, t * 128:(t + 1) * 128].rearrange(
                "(dc p) j -> p dc j", p=128)
            hxv = hx[:].rearrange("p (dc j) -> p dc j", j=128)
            qc = DCH // 4
            nc.sync.dma_start(hxv[:, q * qc:(q + 1) * qc, :],
                              src[:, q * qc:(q + 1) * qc, :])

        # per-tile wave state
        wv_tile = {}    # (t, half) -> psum tile

        def sims_mm(t, half, k):
            key = (t, half)
            if key not in wv_tile:
                wv_tile[key] = wv_p.tile([128, CH], f32, tag="wv",
                                         name=f"wv{t}_{half}")
            nc.tensor.matmul(wv_tile[key][:],
                             hx_tiles[t][:, k * 128:(k + 1) * 128],
                             am_tiles[(half, k)][:],
                             start=(k == 0), stop=(k == DCH - 1))

        def hxsq_piece(t, g):
            """ACT: square 2 chunks of hx[t] (chunks 2g, 2g+1) to bf16."""
            p = sqh_p.tile([128, 256], bf16, tag="sqh", name=f"sqh{t}_{g}")
            nc.scalar.square(p[:], hx_tiles[t][:, g * 256:(g + 1) * 256]
                             .bitcast(f32))
            return p

        def hvs_norm_mm(t, k, piece):
            nc.tensor.matmul(nrm[:, 8 + t:9 + t],
                             piece[:, (k % 2) * 128:(k % 2 + 1) * 128],
                             ones_t[:], start=(k == 0), stop=(k == DCH - 1))

        def am_norm_chunk(half, k):
            """DVE square of am chunk + 4 reversed norm matmuls (ap=1)."""
            sq = sqa_p.tile([128, CH], bf16, tag="sqa", name=f"sqa{half}_{k}")
            nc.vector.tensor_mul(sq[:], am_tiles[(half, k)][:].bitcast(f32),
                                 am_tiles[(half, k)][:].bitcast(f32))
            for b in range(4):
                nc.tensor.matmul(nrm[:, 4 * half + b:4 * half + b + 1],
                                 sq[:, b * 128:(b + 1) * 128], ones_t[:],
                                 start=(k == 0), stop=(k == DCH - 1))

        def am_norm_finalize(half):
            """norm^2 cols [c,1] -> inv_cb[:, half*CH:(half+1)*CH]."""
            amn = ep_p.tile([128, 4], f32, tag="amn", name=f"amn{half}")
            nc.scalar.sqrt(amn[:], nrm[:, 4 * half:4 * half + 4])
            nc.vector.tensor_scalar_max(amn[:], amn[:], EPS)
            invq = ep_p.tile([128, 4], f32, tag="invq", name=f"invq{half}")
            nc.vector.reciprocal(invq[:], amn[:])
            # transpose each column [128c, 1] -> [1, 128c] via matmul vs
            # identity, landing all 4 blocks in one [1, CH] row
            trp = tr_p.tile([1, CH], f32, tag="tr", name=f"tr{half}")
            for b in range(4):
                nc.tensor.matmul(trp[:, b * 128:(b + 1) * 128],
                                 invq[:, b:b + 1], ident_t[:],
                                 start=True, stop=True)
            inv_row = ep_p.tile([1, CH], f32, tag="invrow",
                                name=f"invrow{half}", bufs=1)
            nc.scalar.copy(inv_row[:], trp[:])
            # exact fp32 ones outer-product broadcast to all partitions
            bcp = bc_p.tile([128, CH], f32, tag="bc", name=f"bc{half}")
            nc.tensor.matmul(bcp[:], ones_ct[:], inv_row[:],
                             start=True, stop=True)
            nc.scalar.copy(inv_cb[:, half * CH:(half + 1) * CH], bcp[:])

        def epilogue_A(t):
            wv = wv_tile.pop((t, 0))
            sc = sc_p.tile([128, CH], f32, tag="sc", name=f"scA{t}")
            nc.vector.tensor_mul(sc[:], wv[:], inv_cb[:, 0:CH])
            mxA = mx_acc[:, 8 * t:8 * t + 8]
            nc.vector.max(out=mxA, in_=sc[:])
            ix = ep_p.tile([128, 8], u32, tag="ix", name=f"ixA{t}")
            nc.vector.max_index(out=ix[:], in_max=mxA, in_values=sc[:])
            nc.vector.tensor_copy(preds_acc[:, t:t + 1], ix[:, 0:1])
            # eta (classes 0,1 live in the A half)
            d01 = ep_p.tile([128, 1], f32, tag="d01", name=f"d01{t}")
            nc.vector.tensor_sub(d01[:], sc[:, 1:2], sc[:, 0:1])
            nrm_t = ep_p.tile([128, 1], f32, tag="nrmt", name=f"nrmt{t}")
            nc.scalar.sqrt(nrm_t[:], nrm[:, 8 + t:9 + t])
            nc.vector.tensor_scalar_max(nrm_t[:], nrm_t[:], EPS)
            inv_n = ep_p.tile([128, 1], f32, tag="invn", name=f"invn{t}")
            nc.vector.reciprocal(inv_n[:], nrm_t[:])
            nc.vector.tensor_mul(d01[:], d01[:], inv_n[:])
            nc.vector.tensor_scalar(
                out=eta_acc[:, t:t + 1], in0=d01[:], scalar1=0.25, scalar2=0.5,
                op0=mybir.AluOpType.mult, op1=mybir.AluOpType.add)

        def epilogue_B(t):
            wv = wv_tile.pop((t, 1))
            sc = sc_p.tile([128, CH], f32, tag="sc", name=f"scB{t}")
            nc.vector.tensor_mul(sc[:], wv[:], inv_cb[:, CH:C])
            mxB = ep_p.tile([128, 8], f32, tag="mxB", name=f"mxB{t}")
            nc.vector.max(out=mxB[:], in_=sc[:])
            ix = ep_p.tile([128, 8], u32, tag="ix", name=f"ixB{t}")
            nc.vector.max_index(out=ix[:], in_max=mxB[:], in_values=sc[:])
            ixb = ep_p.tile([128, 1], u32, tag="ixb", name=f"ixb{t}")
            nc.vector.tensor_scalar_add(ixb[:], ix[:, 0:1], CH)
            mask = ep_p.tile([128, 1], u32, tag="mask", name=f"mask{t}")
            nc.vector.tensor_tensor(mask[:], mxB[:, 0:1],
                                    mx_acc[:, 8 * t:8 * t + 1],
                                    mybir.AluOpType.is_gt)
            nc.vector.copy_predicated(preds_acc[:, t:t + 1], mask[:], ixb[:])

        # ================= emission =================
        # ---- window A: interleave hx quarters + amA chunks; wave tiles
        # accumulate chunk-paced with catch-up as their hx arrives ----
        emitted_pieces = {}

        def emit_wave_chunk(t, k):
            g = k // 2
            if (t, g) not in emitted_pieces:
                emitted_pieces[(t, g)] = hxsq_piece(t, g)
            sims_mm(t, 0, k)
            hvs_norm_mm(t, k, emitted_pieces[(t, g)])

        for k in range(DCH):
            if k < 16:
                # quarter q of tile t goes out before amA chunk 4q+t
                dma_hx_quarter(k % 4, k // 4)
            dma_am(0, k)
            am_norm_chunk(0, k)
            # tile t joins at chunk t (its first hx quarter has landed)
            for t in range(WAVE):
                if t == k:
                    for kk in range(k + 1):
                        emit_wave_chunk(t, kk)
                elif t < k:
                    emit_wave_chunk(t, k)

        # ---- A finalize + wave A epilogues ----
        am_norm_finalize(0)
        for t in range(WAVE):
            epilogue_A(t)

        # ---- window B: amB chunks, wave B halves chunk-paced ----
        for k in range(DCH):
            dma_am(1, k)
            am_norm_chunk(1, k)
            for t in range(WAVE):
                sims_mm(t, 1, k)

        # prefetch next hx tiles (quarters so the first chunks land ~1.5us
        # after the reused slot frees, instead of a full 6us tile)
        for q in range(4):
            dma_hx_quarter(4, q)
        for q in range(4):
            dma_hx_quarter(5, q)

        am_norm_finalize(1)
        for t in range(WAVE):
            epilogue_B(t)

        # ---- steady tiles ----
        for t in range(WAVE, NT):
            if t + 2 < NT:
                for q in range(4):
                    dma_hx_quarter(t + 2, q)
            for k in range(DCH):
                g = k // 2
                if (t, g) not in emitted_pieces:
                    emitted_pieces[(t, g)] = hxsq_piece(t, g)
                sims_mm(t, 0, k)
                hvs_norm_mm(t, k, emitted_pieces[(t, g)])
            epilogue_A(t)
            for k in range(DCH):
                sims_mm(t, 1, k)
            epilogue_B(t)

        nc.sync.dma_start(preds_o[:], preds_acc[:])
        nc.sync.dma_start(eta_o[:], eta_acc[:])

    _split_multiwait(nc)
    return nc


_CACHE = {}


def kernel(hvs: np.ndarray, am: np.ndarray):
    hvs = np.asarray(hvs, dtype=np.float32)
    am = np.asarray(am, dtype=np.float32)
    assert hvs.shape == (N_FULL, D) and am.shape == (C, D)

    if "nc" not in _CACHE:
        _CACHE["nc"] = build_nc()
    nc = _CACHE["nc"]

    amT = np.ascontiguousarray(am.T)                      # [D, C]
    import ml_dtypes
    ones_b = np.ones((128, 1), dtype=ml_dtypes.bfloat16)
    ones_r = np.ones((1, 128), dtype=np.float32)
    ident = np.eye(128, dtype=np.float32)

    in_maps = []
    for r in range(N_CORES):
        shard = hvs[r * NS:(r + 1) * NS]                  # [NS, D]
        hvsT = np.ascontiguousarray(shard.T)              # [D, NS]
        in_maps.append({"hvsT": hvsT, "amT": amT, "ones_b": ones_b,
                        "ones_r": ones_r, "ident": ident})

    res = run_bass_kernel_spmd(nc, in_maps, core_ids=list(range(N_CORES)))

    preds = np.empty(N_FULL, dtype=np.int32)
    eta = np.empty(N_FULL, dtype=np.float32)
    for r in range(N_CORES):
        p = res.results[r]["preds"]                       # [128, NT] u32
        e = res.results[r]["eta"]                         # [128, NT] f32
        preds[r * NS:(r + 1) * NS] = p.T.ravel().astype(np.int32)
        eta[r * NS:(r + 1) * NS] = e.T.ravel()
    return preds, eta
